# revision 9
# baseline (speedup 1.0000x reference)
"""
DeepseekV2-MLA fused qkv-a projection kernel for one TRN2 chip (8 NeuronCores).

Computes, for hidden_states [8192, 7168] and w_qkv_a [7168, 2112]:
    qkv   = hidden_states @ w_qkv_a              (bf16 compute, fp32 PSUM accum)
    q_c   = qkv[:, :1536]   -> RMSNorm -> per-128-group fp8-style quant
    kv_c  = qkv[:, 1536:2048] -> RMSNorm
    k_pe  = qkv[:, 2048:2112] (passthrough)

Sharding: pure data-parallel over the token dim M (8 shards of 1024 rows).
Each core reads its hidden shard + the full weight; the RMSNorm/quant
epilogue is row-local, so there are no collectives. hidden^T tiles (the
matmul stationary operand needs K on partitions) are produced on-chip with
PE transposes, just in time, so nothing serializes at startup.

Outputs gathered host-side and returned as the reference's 4-tuple.
"""

import numpy as np

# ---- problem constants (hardcoded; kernel.py must be self-contained) ----
M_FULL, H = 8192, 7168
NQ, NKV, NR = 1536, 512, 64          # q_lora, kv_lora, rope widths
NTOT = NQ + NKV + NR                 # 2112
N_CORES = 8
M = M_FULL // N_CORES                # 1024 rows per core
P = 128
NK = H // P                          # 56 contraction tiles
MT = M // P                          # 8 row tiles per core
GROUP = 128
NGROUPS = NQ // GROUP                # 12
FP8_MAX = 448.0
EPS = 1e-6
CW = 256                             # N-chunk width (weight residency unit)
STAGE = 512                          # hidden load stage width
N_QCH = NQ // CW                     # 6 q chunks
# (col0, width, kind): 6 q chunks, 2 kv chunks, 1 rope chunk
CHUNKS = (
    [(i * CW, CW, "q") for i in range(N_QCH)]
    + [(NQ, CW, "kv"), (NQ + CW, CW, "kv"), (NQ + NKV, NR, "rope")]
)

_CACHE = {}


def _build_nc():
    import concourse.bass as bass
    import concourse.mybir as mybir
    import concourse.tile as tile
    from concourse.masks import make_identity
    from contextlib import ExitStack

    f32 = mybir.dt.float32
    bf16 = mybir.dt.bfloat16
    Alu = mybir.AluOpType
    Act = mybir.ActivationFunctionType

    nc = bass.Bass("TRN2", target_bir_lowering=False, debug=False)

    hs = nc.declare_dram_parameter("hidden_states", [M, H], f32, isOutput=False).ap()
    w = nc.declare_dram_parameter("w_qkv_a", [H, NTOT], f32, isOutput=False).ap()
    qlnw = nc.declare_dram_parameter(
        "q_a_layernorm_weight", [NQ], f32, isOutput=False
    ).ap()
    kvlnw = nc.declare_dram_parameter(
        "kv_a_layernorm_weight", [NKV], f32, isOutput=False
    ).ap()
    o_qq = nc.declare_dram_parameter("q_c_quant", [M, NQ], f32, isOutput=True).ap()
    o_qs = nc.declare_dram_parameter("q_c_scale", [M, NGROUPS], f32, isOutput=True).ap()
    o_kv = nc.declare_dram_parameter("kv_c_normed", [M, NKV], f32, isOutput=True).ap()
    o_kpe = nc.declare_dram_parameter("k_pe", [M, NR], f32, isOutput=True).ap()

    # weight viewed with K split onto partitions: [128, 56, 2112]
    w_r = w.rearrange("(ko p) n -> p ko n", p=P)

    with tile.TileContext(nc) as tc, ExitStack() as ctx:
        singles = ctx.enter_context(tc.tile_pool(name="singles", bufs=1))
        stage_pool = ctx.enter_context(tc.tile_pool(name="stage", bufs=3))
        wpool = ctx.enter_context(tc.tile_pool(name="wpool", bufs=2))
        qt_pool = ctx.enter_context(tc.tile_pool(name="qt", bufs=3))
        sq_pool = ctx.enter_context(tc.tile_pool(name="sq", bufs=2))
        out_pool = ctx.enter_context(tc.tile_pool(name="outs", bufs=2))
        fin_pool = ctx.enter_context(tc.tile_pool(name="fin", bufs=2))
        psum_mm = ctx.enter_context(tc.tile_pool(name="psum_mm", bufs=3, space="PSUM"))
        psum_tr = ctx.enter_context(tc.tile_pool(name="psum_tr", bufs=3, space="PSUM"))
        dram_pool = ctx.enter_context(tc.tile_pool(name="dram", bufs=1, space="DRAM"))

        # ---- constants ----
        ident = singles.tile([P, P], bf16, tag="ident")
        make_identity(nc, ident)
        eps_t = singles.tile([P, 1], f32, tag="eps")
        nc.vector.memset(eps_t, EPS)
        qw_b = singles.tile([P, NQ], f32, tag="qw_b")
        nc.sync.dma_start(out=qw_b, in_=qlnw.partition_broadcast(P))
        kvw_b = singles.tile([P, NKV], f32, tag="kvw_b")
        nc.sync.dma_start(out=kvw_b, in_=kvlnw.partition_broadcast(P))

        # ---- persistent per-m state ----
        hT = [singles.tile([P, NK, P], bf16, tag=f"hT{m}", name=f"hT{m}") for m in range(MT)]
        kvt = [singles.tile([P, NKV], bf16, tag=f"kvt{m}", name=f"kvt{m}") for m in range(MT)]
        ssq_q = [singles.tile([P, N_QCH], f32, tag=f"ssq_q{m}", name=f"ssq_q{m}") for m in range(MT)]
        ssq_kv = [singles.tile([P, 2], f32, tag=f"ssq_kv{m}", name=f"ssq_kv{m}") for m in range(MT)]
        amax_q = [singles.tile([P, NGROUPS], f32, tag=f"amax{m}", name=f"amax{m}") for m in range(MT)]
        qts = [dram_pool.tile([P, NQ], f32, tag=f"qts{m}", name=f"qts{m}") for m in range(MT)]

        def load_w_chunk(c):
            col0, cw, _ = CHUNKS[c]
            wt = wpool.tile([P, NK, CW], bf16, tag="wchunk")
            for g in range(7):  # 7 groups of 8 k-tiles -> ~1MB casting DMAs
                nc.gpsimd.dma_start(
                    out=wt[:, g * 8 : (g + 1) * 8, :cw],
                    in_=w_r[:, g * 8 : (g + 1) * 8, col0 : col0 + cw],
                )
            return wt

        def mm_chunk(c, m, wt):
            _, cw, _ = CHUNKS[c]
            pm = psum_mm.tile([P, CW], mybir.dt.float32, tag="pm")
            for k in range(NK):
                nc.tensor.matmul(
                    pm[:, :cw],
                    hT[m][:, k, :],
                    wt[:, k, :cw],
                    start=(k == 0),
                    stop=(k == NK - 1),
                )
            return pm

        def epilogue(c, m, pm):
            col0, cw, kind = CHUNKS[c]
            rows = slice(m * P, (m + 1) * P)
            if kind == "rope":
                t = out_pool.tile([P, NR], f32, tag="rope")
                nc.any.tensor_copy(out=t, in_=pm[:, :NR])
                nc.sync.dma_start(out=o_kpe[rows, :], in_=t)
                return
            if kind == "q":
                qi = c
                # sum(x^2) partial for this chunk (ACT: square + accumulate)
                sq = sq_pool.tile([P, CW], bf16, tag="sq")
                nc.scalar.activation(
                    out=sq[:, :cw],
                    in_=pm[:, :cw],
                    func=Act.Square,
                    accum_out=ssq_q[m][:, qi : qi + 1],
                )
                # t = x * ln_weight (into bf16), amax per 128-group, spill
                t = qt_pool.tile([P, CW], f32, tag="qt")
                nc.vector.tensor_tensor(
                    t[:, :cw], pm[:, :cw], qw_b[:, col0 : col0 + cw], Alu.mult
                )
                ng = cw // GROUP
                nc.vector.tensor_reduce(
                    out=amax_q[m][:, qi * ng : (qi + 1) * ng],
                    in_=t[:, :cw].rearrange("p (g d) -> p g d", g=ng),
                    axis=mybir.AxisListType.X,
                    op=Alu.max,
                    apply_absolute_value=True,
                )
                nc.sync.dma_start(out=qts[m][:, col0 : col0 + cw], in_=t[:, :cw])
            else:  # kv
                ki = c - N_QCH
                sq = sq_pool.tile([P, CW], bf16, tag="sq")
                nc.scalar.activation(
                    out=sq[:, :cw],
                    in_=pm[:, :cw],
                    func=Act.Square,
                    accum_out=ssq_kv[m][:, ki : ki + 1],
                )
                nc.vector.tensor_tensor(
                    kvt[m][:, ki * CW : ki * CW + cw],
                    pm[:, :cw],
                    kvw_b[:, ki * CW : ki * CW + cw],
                    Alu.mult,
                )

        def final_q(m):
            rows = slice(m * P, (m + 1) * P)
            ssqt = fin_pool.tile([P, 1], f32, tag="ssqt")
            nc.vector.reduce_sum(out=ssqt, in_=ssq_q[m], axis=mybir.AxisListType.X)
            std = fin_pool.tile([P, 1], f32, tag="std")
            nc.scalar.activation(
                out=std, in_=ssqt, func=Act.Sqrt, bias=eps_t, scale=1.0 / NQ
            )
            rq = fin_pool.tile([P, 1], f32, tag="rq")
            nc.vector.reciprocal(out=rq, in_=std)
            # scale = max(amax * rq / 448, 1e-12)
            qsc = fin_pool.tile([P, NGROUPS], f32, tag="qsc")
            nc.vector.tensor_scalar(
                out=qsc,
                in0=amax_q[m],
                scalar1=rq[:, 0:1],
                scalar2=1.0 / FP8_MAX,
                op0=Alu.mult,
                op1=Alu.mult,
            )
            nc.vector.tensor_scalar_max(out=qsc, in0=qsc, scalar1=1e-12)
            nc.sync.dma_start(out=o_qs[rows, :], in_=qsc)
            # rs = rq / scale (per group); q_quant = t * rs
            inv = fin_pool.tile([P, NGROUPS], f32, tag="inv")
            nc.vector.reciprocal(out=inv, in_=qsc)
            rs = fin_pool.tile([P, NGROUPS], f32, tag="rs")
            nc.vector.tensor_scalar_mul(out=rs, in0=inv, scalar1=rq[:, 0:1])
            for j in range(NQ // STAGE):  # 3 x 512-col blocks
                rb = fin_pool.tile([P, STAGE], f32, tag="rb")
                nc.sync.dma_start(
                    out=rb, in_=qts[m][:, j * STAGE : (j + 1) * STAGE]
                )
                gpj = STAGE // GROUP  # 4 groups per block
                qq = fin_pool.tile([P, gpj, GROUP], f32, tag="qq")
                nc.vector.tensor_tensor(
                    qq,
                    rb.rearrange("p (g d) -> p g d", g=gpj),
                    rs[:, j * gpj : (j + 1) * gpj, None].to_broadcast((P, gpj, GROUP)),
                    Alu.mult,
                )
                nc.sync.dma_start(
                    out=o_qq[rows, j * STAGE : (j + 1) * STAGE], in_=qq
                )

        def final_kv(m):
            rows = slice(m * P, (m + 1) * P)
            ssqt = fin_pool.tile([P, 1], f32, tag="kssqt")
            nc.vector.reduce_sum(out=ssqt, in_=ssq_kv[m], axis=mybir.AxisListType.X)
            std = fin_pool.tile([P, 1], f32, tag="kstd")
            nc.scalar.activation(
                out=std, in_=ssqt, func=Act.Sqrt, bias=eps_t, scale=1.0 / NKV
            )
            rkv = fin_pool.tile([P, 1], f32, tag="rkv")
            nc.vector.reciprocal(out=rkv, in_=std)
            kvo = fin_pool.tile([P, NKV], f32, tag="kvo")
            nc.vector.tensor_scalar_mul(out=kvo, in0=kvt[m], scalar1=rkv[:, 0:1])
            nc.sync.dma_start(out=o_kv[rows, :], in_=kvo)

        # ================= main schedule =================
        wt0 = load_w_chunk(0)
        # Phase A: per row-tile, build hidden^T via PE transposes, then chunk-0
        for m in range(MT):
            for s in range(H // STAGE):  # 14 stages of 512 columns
                stg = stage_pool.tile([P, STAGE], bf16, tag="stage")
                # SWDGE cast-DMA: DRAM f32 -> SBUF bf16
                nc.gpsimd.dma_start(
                    out=stg,
                    in_=hs[m * P : (m + 1) * P, s * STAGE : (s + 1) * STAGE],
                )
                ptr = psum_tr.tile([P, 4, P], bf16, tag="ptr")
                for t in range(4):
                    nc.tensor.transpose(
                        ptr[:, t, :], stg[:, t * P : (t + 1) * P], ident
                    )
                nc.any.tensor_copy(out=hT[m][:, s * 4 : (s + 1) * 4, :], in_=ptr)
            epilogue(0, m, mm_chunk(0, m, wt0))
        # Phases B: remaining chunks over resident hidden^T
        for c in range(1, len(CHUNKS)):
            wt = load_w_chunk(c)
            for m in range(MT):
                epilogue(c, m, mm_chunk(c, m, wt))
                if c == N_QCH - 1:
                    final_q(m)
                if c == N_QCH + 1:
                    final_kv(m)
    return nc


def _fix_sync_waits(nc):
    """Split instructions carrying more than one sync-wait.

    The walrus in this toolchain encodes at most one sync-wait command per
    instruction (verified empirically for Drain/DMACopy/Matmult structs);
    Tile emits up to ~20. Excess waits move onto injected wait-only Drain
    instructions on the same engine, placed immediately before the original
    (same-engine program order => identical semantics).
    """
    from concourse import mybir

    def limit_of(ins):
        return 1

    for f in nc.m.functions:
        for bb in f.blocks:
            out = []
            for ins in bb.instructions:
                si = getattr(ins, "sync_info", None)
                limit = limit_of(ins)
                if (
                    limit is not None
                    and si is not None
                    and si.on_wait
                    and len(si.on_wait) > limit
                ):
                    waits = list(si.on_wait)
                    excess, keep = waits[: len(waits) - limit], waits[-limit:]
                    for wchunk in [excess[i : i + 1] for i in range(len(excess))]:
                        d = mybir.InstDrain(
                            name=nc.get_next_instruction_name(),
                            ins=[],
                            outs=[],
                            bass_is_fusable=False,
                        )
                        d.engine = ins.engine
                        d.sync_info = mybir.SyncInfo(on_wait=wchunk, on_update=[])
                        out.append(d)
                    si.on_wait = keep
                out.append(ins)
            bb.instructions[:] = out


def _get_nc():
    if "nc" not in _CACHE:
        nc = _build_nc()
        _fix_sync_waits(nc)
        _CACHE["nc"] = nc
    return _CACHE["nc"]


def kernel(hidden_states, w_qkv_a, q_a_layernorm_weight, kv_a_layernorm_weight):
    from concourse.bass_utils import run_bass_kernel_spmd

    hidden_states = np.ascontiguousarray(hidden_states, dtype=np.float32)
    w_qkv_a = np.ascontiguousarray(w_qkv_a, dtype=np.float32)
    q_a_layernorm_weight = np.ascontiguousarray(q_a_layernorm_weight, dtype=np.float32)
    kv_a_layernorm_weight = np.ascontiguousarray(
        kv_a_layernorm_weight, dtype=np.float32
    )

    nc = _get_nc()
    core_ids = list(range(N_CORES))
    in_maps = [
        {
            "hidden_states": hidden_states[i * M : (i + 1) * M],
            "w_qkv_a": w_qkv_a,
            "q_a_layernorm_weight": q_a_layernorm_weight,
            "kv_a_layernorm_weight": kv_a_layernorm_weight,
        }
        for i in core_ids
    ]
    res = run_bass_kernel_spmd(nc, in_maps, core_ids).results
    q_c_quant = np.concatenate([res[i]["q_c_quant"] for i in core_ids], axis=0)
    q_c_scale = np.concatenate([res[i]["q_c_scale"] for i in core_ids], axis=0)
    kv_c_normed = np.concatenate([res[i]["kv_c_normed"] for i in core_ids], axis=0)
    k_pe = np.concatenate([res[i]["k_pe"] for i in core_ids], axis=0)
    return q_c_quant, q_c_scale, kv_c_normed, k_pe
